# revision 8
# baseline (speedup 1.0000x reference)
"""Trainium2 Bass kernel for a fused multi-head attention block.

Reference computation (B=2, S=2048, H=1024, NH=16, HD=64):
    qh/kh/vh = (x @ W + b) per head
    energy   = qh @ kh^T  (full S x S per head)
    attn     = softmax(where(mask==0, -1e9, energy) / sqrt(H))
    out      = attn @ vh
    y        = out @ Wfc + bfc + q (residual)
    return LayerNorm(y) * gamma + beta

Sharding: data-parallel over batch (2 groups of 4 cores) x tensor-parallel
over heads (4 heads per core). Wq/Wk/Wv column-sharded, Wfc row-sharded,
ReduceScatter(add) over each 4-core group after fc, then per-core
residual+LayerNorm on its 512-row output slice.

Per-core kernel layout choices:
  * q/k projections produce TRANSPOSED activations qh^T/kh^T [256, S] so the
    scores matmul (contraction over head dim) can run directly from SBUF.
  * scores are computed transposed: energy^T[k, q] tiles, so exp can be
    applied on the PSUM tile and the softmax denominator comes for free via
    an appended ones-column in the attn@V stationary operand (vext).
  * the masked softmax is multiplicative: P = exp(energy/32) * maskT, which
    is exactly equivalent to the reference's -1e9 additive mask since
    exp(-1e9/32) == 0 in fp32.
  * attn@V accumulates out^T[d|1, q] in PSUM over k-tiles; row 64 holds the
    denominator. The divide is done as reciprocal + broadcast-multiply
    (broadcast via a stride-0 DRAM DMA).
"""

import numpy as np
import ml_dtypes

import concourse.bass as bass
import concourse.mybir as mybir
from concourse import bacc, tile
from concourse.bass_utils import run_bass_kernel_spmd

B, S, H, NH = 2, 2048, 1024, 16
HD = H // NH                  # 64
NCORES = 8
TPG = 4                       # cores per tensor-parallel group
HPC = NH // TPG               # 4 heads per core
DC = HPC * HD                 # 256 head-dims per core
SR = S // TPG                 # 512 output rows per core
INV_SCALE = 1.0 / float(H) ** 0.5   # 1/32
EPS = 1e-5

FP = mybir.dt.float32
BF = mybir.dt.bfloat16
F32 = np.float32
BF16 = ml_dtypes.bfloat16

KT = H // 128                 # 8 contraction tiles for projections
ST = S // 128                 # 16 seq tiles
QC = S // 512                 # 4 q-chunks of 512
RT = SR // 128                # 4 row tiles in the final phase
E1 = HD + 1                   # 65: head dims + ones column

ts = bass.ts
AF = mybir.ActivationFunctionType
ALU = mybir.AluOpType


def _build_nc():
    nc = bacc.Bacc(
        "TRN2",
        target_bir_lowering=False,
        debug=False,
        num_devices=NCORES,
    )

    # ---- per-core DRAM I/O ----
    qT = nc.dram_tensor("qT", [H, S], BF, kind="ExternalInput")
    kTt = nc.dram_tensor("kTt", [H, S], BF, kind="ExternalInput")
    vT = nc.dram_tensor("vT", [H, S], BF, kind="ExternalInput")
    maskT = nc.dram_tensor("maskT", [S, S], BF, kind="ExternalInput")
    wq = nc.dram_tensor("wq", [H, DC], BF, kind="ExternalInput")
    wk = nc.dram_tensor("wk", [H, DC], BF, kind="ExternalInput")
    wv = nc.dram_tensor("wv", [H, DC], BF, kind="ExternalInput")
    wfc = nc.dram_tensor("wfc", [DC, H], BF, kind="ExternalInput")
    bq = nc.dram_tensor("bq", [DC, 1], FP, kind="ExternalInput")
    bk = nc.dram_tensor("bk", [DC, 1], FP, kind="ExternalInput")
    bv = nc.dram_tensor("bv", [1, DC], FP, kind="ExternalInput")
    resid = nc.dram_tensor("resid", [SR, H], FP, kind="ExternalInput")
    gamma = nc.dram_tensor("gamma", [1, H], FP, kind="ExternalInput")
    beta = nc.dram_tensor("beta", [1, H], FP, kind="ExternalInput")
    out = nc.dram_tensor("out", [SR, H], FP, kind="ExternalOutput")

    with tile.TileContext(nc) as tc:
        with (
            tc.tile_pool(name="const", bufs=1) as cpool,
            tc.tile_pool(name="stream", bufs=1) as spool,
            tc.tile_pool(name="mask", bufs=3) as mpool,
            tc.tile_pool(name="work", bufs=3) as wpool,
            tc.tile_pool(name="epi", bufs=2) as epool,
            tc.tile_pool(name="fin", bufs=4) as fpool,
            tc.tile_pool(name="psum", bufs=1, space="PSUM") as ppool,
            tc.tile_pool(name="psA", bufs=2, space="PSUM") as ppoolA,
            tc.tile_pool(name="dram", bufs=1, space="DRAM") as dpool,
            tc.tile_pool(name="dram2", bufs=2, space="DRAM") as dpool2,
        ):
            # ---------- constants ----------
            w_sb = {}
            for name, dram in (("wq", wq), ("wk", wk), ("wv", wv)):
                tiles = []
                for kt in range(KT):
                    t = cpool.tile([128, DC], BF, tag=f"{name}{kt}")
                    nc.sync.dma_start(out=t[:], in_=dram[ts(kt, 128), :])
                    tiles.append(t)
                w_sb[name] = tiles
            wfc_sb = []
            for dg in range(2):
                t = cpool.tile([128, H], BF, tag=f"wfc{dg}")
                nc.sync.dma_start(out=t[:], in_=wfc[ts(dg, 128), :])
                wfc_sb.append(t)
            bias_sb = {}
            for name, dram in (("bq", bq), ("bk", bk)):
                tiles = []
                for nt in range(2):
                    t = cpool.tile([128, 1], FP, tag=f"{name}{nt}")
                    nc.sync.dma_start(out=t[:], in_=dram[ts(nt, 128), :])
                    tiles.append(t)
                bias_sb[name] = tiles
            # broadcast rows loaded via stride-0 DRAM reads
            bvB = cpool.tile([128, DC], FP, tag="bvB")
            nc.sync.dma_start(out=bvB[:], in_=bv[:].broadcast_to([128, DC]))
            gammaB = cpool.tile([128, H], FP, tag="gammaB")
            nc.sync.dma_start(out=gammaB[:], in_=gamma[:].broadcast_to([128, H]))
            betaB = cpool.tile([128, H], FP, tag="betaB")
            nc.sync.dma_start(out=betaB[:], in_=beta[:].broadcast_to([128, H]))

            # ---------- q/k projections (transposed outputs [DC, S]) ----------
            qhT_sb = [cpool.tile([128, S], BF, tag=f"qhT{nt}", name=f"qhT{nt}") for nt in range(2)]
            khT_sb = [cpool.tile([128, S], BF, tag=f"khT{nt}", name=f"khT{nt}") for nt in range(2)]
            for bname, xdram, outsb in (("bq", qT, qhT_sb), ("bk", kTt, khT_sb)):
                wname = "wq" if bname == "bq" else "wk"
                x_tiles = []
                for kt in range(KT):
                    xt = spool.tile([128, S], BF, tag=f"x{kt}")
                    nc.sync.dma_start(out=xt[:], in_=xdram[ts(kt, 128), :])
                    x_tiles.append(xt)
                for nt in range(2):
                    for qc in range(QC):
                        ps = ppoolA.tile([128, 512], FP, tag="A")
                        for kt in range(KT):
                            nc.tensor.matmul(
                                ps[:],
                                lhsT=w_sb[wname][kt][:, ts(nt, 128)],
                                rhs=x_tiles[kt][:, ts(qc, 512)],
                                start=(kt == 0),
                                stop=(kt == KT - 1),
                            )
                        nc.any.tensor_scalar_add(
                            outsb[nt][:, ts(qc, 512)], ps[:], bias_sb[bname][nt][:]
                        )

            # ---------- v projection (natural layout -> vext [S, 4*65]) ----------
            vext_sb = []
            for st in range(ST):
                t = cpool.tile([128, HPC * E1], BF, tag=f"vext{st}")
                vext_sb.append(t)
            vT_tiles = []
            for ht in range(KT):
                xt = spool.tile([128, S], BF, tag=f"x{ht}")
                nc.sync.dma_start(out=xt[:], in_=vT[ts(ht, 128), :])
                vT_tiles.append(xt)
            for st in range(ST):
                ps = ppoolA.tile([128, DC], FP, tag="A")
                for ht in range(KT):
                    nc.tensor.matmul(
                        ps[:],
                        lhsT=vT_tiles[ht][:, ts(st, 128)],
                        rhs=w_sb["wv"][ht][:],
                        start=(ht == 0),
                        stop=(ht == KT - 1),
                    )
                vx = vext_sb[st]
                for h in range(HPC):
                    nc.vector.memset(vx[:, h * E1 + HD : h * E1 + E1], 1.0)
                v3 = vx.rearrange("p (h e) -> p h e", e=E1)[:, :, 0:HD]
                p3 = ps.rearrange("p (h e) -> p h e", e=HD)
                b3 = bvB.rearrange("p (h e) -> p h e", e=HD)
                nc.vector.tensor_add(v3, p3, b3)

            # ---------- attention: head pairs x q-halves ----------
            outT_sc = [cpool.tile([128, S], BF, tag=f"oT{nt}", name=f"oT{nt}") for nt in range(2)]
            for hp in range(2):
                for half in range(2):
                    q0 = 1024 * half
                    Bts = [
                        ppool.tile([E1, 1024], FP, tag=f"attB{hh}", name=f"attB{hh}")
                        for hh in range(2)
                    ]
                    for kj in range(ST):
                        mt = mpool.tile([128, 1024], BF, tag="mask")
                        nc.sync.dma_start(
                            out=mt[:], in_=maskT[ts(kj, 128), q0 : q0 + 1024]
                        )
                        for hh in range(2):
                            hb = 64 * hh
                            A = ppoolA.tile([128, 1024], FP, tag="A")
                            for c in range(2):
                                nc.tensor.matmul(
                                    A[:, ts(c, 512)],
                                    lhsT=khT_sb[hp][hb : hb + 64, ts(kj, 128)],
                                    rhs=qhT_sb[hp][hb : hb + 64, q0 + 512 * c : q0 + 512 * (c + 1)],
                                    start=True,
                                    stop=True,
                                )
                            P = wpool.tile([128, 1024], BF, tag="P")
                            nc.scalar.activation(P[:], A[:], AF.Exp, scale=INV_SCALE)
                            Pm = wpool.tile([128, 1024], BF, tag="Pm")
                            nc.vector.tensor_mul(Pm[:], P[:], mt[:])
                            h = 2 * hp + hh
                            for c in range(2):
                                nc.tensor.matmul(
                                    Bts[hh][:, ts(c, 512)],
                                    lhsT=vext_sb[kj][:, h * E1 : (h + 1) * E1],
                                    rhs=Pm[:, ts(c, 512)],
                                    start=(kj == 0),
                                    stop=(kj == ST - 1),
                                )
                    # epilogue: divide rows 0..63 by the ones-row (denominator)
                    for hh in range(2):
                        hb = 64 * hh
                        dn65 = epool.tile([E1, 1024], FP, tag="dn65")
                        nc.vector.tensor_copy(dn65[64:65, :], Bts[hh][64:65, :])
                        dnP = epool.tile([128, 8], FP, tag="dnP")
                        nc.sync.dma_start(out=dnP[:], in_=dn65[64:65, :])
                        rcP = epool.tile([128, 8], FP, tag="rcP")
                        nc.vector.reciprocal(rcP[:], dnP[:])
                        rdram = dpool2.tile([1, 1024], FP, tag="rdram")
                        nc.sync.dma_start(out=rdram[:], in_=rcP[:])
                        rb = epool.tile([64, 1024], FP, tag="rb")
                        nc.sync.dma_start(
                            out=rb[:], in_=rdram[:].broadcast_to([64, 1024])
                        )
                        if hh == 0:
                            nc.vector.tensor_mul(
                                outT_sc[hp][0:64, q0 : q0 + 1024],
                                Bts[hh][0:64, :],
                                rb[:],
                            )
                        else:
                            osc = epool.tile([64, 1024], BF, tag="osc")
                            nc.vector.tensor_mul(osc[:], Bts[hh][0:64, :], rb[:])
                            nc.sync.dma_start(
                                out=outT_sc[hp][64:128, q0 : q0 + 1024], in_=osc[:]
                            )

            # ---------- fc partial + chunked reduce-scatter ----------
            # y_part chunk i covers s-rows [512i, 512(i+1)); after each chunk's
            # four s-tiles finish, a ReduceScatter over the 4-core group hands
            # this core rows [512i+128r, 512i+128(r+1)) (r = group rank).
            y_chunks = [
                dpool.tile([SR, H], BF, tag=f"y_part{i}", name=f"y_part{i}")
                for i in range(RT)
            ]
            z_chunks = [
                dpool.tile([128, H], BF, tag=f"z{i}", name=f"z{i}")
                for i in range(RT)
            ]
            for st in range(ST):
                ps = ppoolA.tile([128, H], FP, tag="A")
                for dg in range(2):
                    for hc in range(2):
                        nc.tensor.matmul(
                            ps[:, ts(hc, 512)],
                            lhsT=outT_sc[dg][:, ts(st, 128)],
                            rhs=wfc_sb[dg][:, ts(hc, 512)],
                            start=(dg == 0),
                            stop=(dg == 1),
                        )
                yb = fpool.tile([128, H], BF, tag="yb")
                nc.any.tensor_copy(yb[:], ps[:])
                nc.sync.dma_start(
                    out=y_chunks[st // 4][ts(st % 4, 128), :], in_=yb[:]
                )
                if st % 4 == 3:
                    nc.gpsimd.collective_compute(
                        "ReduceScatter",
                        ALU.add,
                        replica_groups=[[0, 1, 2, 3], [4, 5, 6, 7]],
                        ins=[y_chunks[st // 4][:]],
                        outs=[z_chunks[st // 4][:]],
                    )

            # ---------- residual + layernorm on own 4x128-row slices ----------
            for rt in range(RT):
                zbf = fpool.tile([128, H], BF, tag="zbf")
                nc.sync.dma_start(out=zbf[:], in_=z_chunks[rt][:])
                rs = fpool.tile([128, H], FP, tag="rs")
                nc.sync.dma_start(out=rs[:], in_=resid[ts(rt, 128), :])
                musum = fpool.tile([128, 1], FP, tag="musum")
                zt = fpool.tile([128, H], FP, tag="zt")
                # y = z + resid, accumulating row sums for the mean
                nc.vector.scalar_tensor_tensor(
                    zt[:], zbf[:], 0.0, rs[:], ALU.add, ALU.add, accum_out=musum[:]
                )
                mu = fpool.tile([128, 1], FP, tag="mu")
                nc.vector.tensor_scalar_mul(mu[:], musum[:], 1.0 / H)
                nc.vector.tensor_scalar_sub(zt[:], zt[:], mu[:])
                ssq = fpool.tile([128, 1], FP, tag="ssq")
                nc.vector.scalar_tensor_tensor(
                    rs[:], zt[:], 0.0, zt[:], ALU.add, ALU.mult, accum_out=ssq[:]
                )
                varp = fpool.tile([128, 1], FP, tag="varp")
                nc.vector.tensor_scalar(
                    varp[:], ssq[:], 1.0 / H, EPS, ALU.mult, ALU.add
                )
                sdev = fpool.tile([128, 1], FP, tag="sdev")
                nc.scalar.activation(sdev[:], varp[:], AF.Sqrt)
                rstd = fpool.tile([128, 1], FP, tag="rstd")
                nc.vector.reciprocal(rstd[:], sdev[:])
                nc.vector.scalar_tensor_tensor(
                    rs[:], zt[:], rstd[:], gammaB[:], ALU.mult, ALU.mult
                )
                ot = fpool.tile([128, H], FP, tag="ot")
                nc.vector.tensor_add(ot[:], rs[:], betaB[:])
                nc.sync.dma_start(out=out[ts(rt, 128), :], in_=ot[:])

    nc.compile()
    return nc


_NC_CACHE = {}


def _get_nc():
    if "nc" not in _NC_CACHE:
        _NC_CACHE["nc"] = _build_nc()
    return _NC_CACHE["nc"]


def _prep_inputs(q, k, v, mask, Wq, bq, Wk, bk, Wv, bv, Wfc, bfc, gamma, beta):
    """Build the 8 per-core input maps on the host (sharding + layout)."""
    q = np.asarray(q, F32)
    k = np.asarray(k, F32)
    v = np.asarray(v, F32)
    mask = np.asarray(mask)
    in_maps = []
    qT_b, kT_b, vT_b, maskT_b = [], [], [], []
    for b in range(B):
        qT_b.append(np.ascontiguousarray(q[b].T).astype(BF16))
        kT_b.append(np.ascontiguousarray(k[b].T).astype(BF16))
        vT_b.append(np.ascontiguousarray(v[b].T).astype(BF16))
        maskT_b.append(np.ascontiguousarray(mask[b, 0].T).astype(BF16))
    Wq_bf, Wk_bf, Wv_bf, Wfc_bf = (
        np.asarray(w, F32).astype(BF16) for w in (Wq, Wk, Wv, Wfc)
    )
    for c in range(NCORES):
        b, g = c // TPG, c % TPG
        cols = slice(g * DC, (g + 1) * DC)
        in_maps.append({
            "qT": qT_b[b],
            "kTt": kT_b[b],
            "vT": vT_b[b],
            "maskT": maskT_b[b],
            "wq": np.ascontiguousarray(Wq_bf[:, cols]),
            "wk": np.ascontiguousarray(Wk_bf[:, cols]),
            "wv": np.ascontiguousarray(Wv_bf[:, cols]),
            "wfc": np.ascontiguousarray(Wfc_bf[cols, :]),
            "bq": np.asarray(bq, F32)[cols].reshape(DC, 1),
            "bk": np.asarray(bk, F32)[cols].reshape(DC, 1),
            "bv": np.asarray(bv, F32)[cols].reshape(1, DC),
            "resid": np.ascontiguousarray(
                np.concatenate(
                    [
                        q[b, 512 * i + 128 * g : 512 * i + 128 * (g + 1)]
                        for i in range(RT)
                    ]
                )
                + np.asarray(bfc, F32)[None, :]
            ),
            "gamma": np.asarray(gamma, F32).reshape(1, H),
            "beta": np.asarray(beta, F32).reshape(1, H),
        })
    return in_maps


_LAST_RUN_S = [None]


def kernel(q, k, v, mask, Wq, bq, Wk, bk, Wv, bv, Wfc, bfc, gamma, beta):
    import time

    nc = _get_nc()
    in_maps = _prep_inputs(
        q, k, v, mask, Wq, bq, Wk, bk, Wv, bv, Wfc, bfc, gamma, beta
    )
    t0 = time.perf_counter()
    res = run_bass_kernel_spmd(nc, in_maps, list(range(NCORES)))
    _LAST_RUN_S[0] = time.perf_counter() - t0
    full = np.empty((B, S, H), F32)
    for c in range(NCORES):
        b, r = c // TPG, c % TPG
        o = res.results[c]["out"]
        for i in range(RT):
            full[b, 512 * i + 128 * r : 512 * i + 128 * (r + 1)] = o[
                128 * i : 128 * (i + 1)
            ]
    return full


# revision 20
# speedup vs baseline: 64.7423x; 64.7423x over previous
"""Trainium2 Bass kernel for a fused multi-head attention block.

Reference computation (B=2, S=2048, H=1024, NH=16, HD=64):
    qh/kh/vh = (x @ W + b) per head
    energy   = qh @ kh^T  (full S x S per head)
    attn     = softmax(where(mask==0, -1e9, energy) / sqrt(H))
    out      = attn @ vh
    y        = out @ Wfc + bfc + q (residual)
    return LayerNorm(y) * gamma + beta

Sharding: data-parallel over batch (2 groups of 4 cores) x tensor-parallel
over heads (4 heads per core). Wq/Wk/Wv column-sharded, Wfc row-sharded,
ReduceScatter(add) over each 4-core group after fc, then per-core
residual+LayerNorm on its 512-row output slice.

Per-core kernel layout choices:
  * q/k projections produce TRANSPOSED activations qh^T/kh^T [256, S] so the
    scores matmul (contraction over head dim) can run directly from SBUF.
  * scores are computed transposed: energy^T[k, q] tiles, so exp can be
    applied on the PSUM tile and the softmax denominator comes for free via
    an appended ones-column in the attn@V stationary operand (vext).
  * the masked softmax is multiplicative: P = exp(energy/32) * maskT, which
    is exactly equivalent to the reference's -1e9 additive mask since
    exp(-1e9/32) == 0 in fp32.
  * attn@V accumulates out^T[d|1, q] in PSUM over k-tiles; row 64 holds the
    denominator. The divide is done as reciprocal + broadcast-multiply
    (broadcast via a stride-0 DRAM DMA).
"""

import numpy as np
import ml_dtypes

import concourse.bass as bass
import concourse.mybir as mybir
from concourse import bacc, tile
from concourse.bass_utils import run_bass_kernel_spmd

B, S, H, NH = 2, 2048, 1024, 16
HD = H // NH                  # 64
NCORES = 8
TPG = 4                       # cores per tensor-parallel group
HPC = NH // TPG               # 4 heads per core
DC = HPC * HD                 # 256 head-dims per core
SR = S // TPG                 # 512 output rows per core
INV_SCALE = 1.0 / float(H) ** 0.5   # 1/32
EPS = 1e-5

FP = mybir.dt.float32
BF = mybir.dt.bfloat16
F32 = np.float32
BF16 = ml_dtypes.bfloat16

KT = H // 128                 # 8 contraction tiles for projections
ST = S // 128                 # 16 seq tiles
QC = S // 512                 # 4 q-chunks of 512
RT = SR // 128                # 4 row tiles in the final phase
E1 = HD + 1                   # 65: head dims + ones column

ts = bass.ts
AF = mybir.ActivationFunctionType
ALU = mybir.AluOpType


def _build_nc():
    nc = bacc.Bacc(
        "TRN2",
        target_bir_lowering=False,
        debug=False,
        num_devices=NCORES,
    )

    # ---- per-core DRAM I/O ----
    qT = nc.dram_tensor("qT", [H, S], BF, kind="ExternalInput")
    kTt = nc.dram_tensor("kTt", [H, S], BF, kind="ExternalInput")
    vT = nc.dram_tensor("vT", [H, S], BF, kind="ExternalInput")
    maskT = nc.dram_tensor("maskT", [S, S], BF, kind="ExternalInput")
    wq = nc.dram_tensor("wq", [H, DC], BF, kind="ExternalInput")
    wk = nc.dram_tensor("wk", [H, DC], BF, kind="ExternalInput")
    wv = nc.dram_tensor("wv", [H, DC], BF, kind="ExternalInput")
    wfc = nc.dram_tensor("wfc", [DC, H], BF, kind="ExternalInput")
    bq = nc.dram_tensor("bq", [DC, 1], FP, kind="ExternalInput")
    bk = nc.dram_tensor("bk", [DC, 1], FP, kind="ExternalInput")
    bv = nc.dram_tensor("bv", [1, DC], FP, kind="ExternalInput")
    resid = nc.dram_tensor("resid", [SR, H], FP, kind="ExternalInput")
    gamma = nc.dram_tensor("gamma", [1, H], FP, kind="ExternalInput")
    beta = nc.dram_tensor("beta", [1, H], FP, kind="ExternalInput")
    out = nc.dram_tensor("out", [SR, H], FP, kind="ExternalOutput")

    with tile.TileContext(nc) as tc:
        with (
            tc.tile_pool(name="const", bufs=1) as cpool,
            tc.tile_pool(name="stream", bufs=2) as spool,
            tc.tile_pool(name="mask", bufs=4) as mpool,
            tc.tile_pool(name="work", bufs=4) as wpool,
            tc.tile_pool(name="epi", bufs=2) as epool,
            tc.tile_pool(name="fin", bufs=2) as fpool,
            tc.tile_pool(name="psum", bufs=1, space="PSUM") as ppool,
            tc.tile_pool(name="psA", bufs=2, space="PSUM") as ppoolA,
            tc.tile_pool(name="dram", bufs=1, space="DRAM") as dpool,
            tc.tile_pool(name="dram2", bufs=2, space="DRAM") as dpool2,
        ):
            # ---------- constants ----------
            # weights are loaded lazily right before the phase that uses them
            # so the first projection's inputs aren't queued behind them
            w_dram = {"wq": wq, "wk": wk, "wv": wv}
            w_sb = {}

            def load_w(name):
                tiles = []
                for kt in range(KT):
                    t = cpool.tile(
                        [128, DC], BF, tag=f"{name}{kt}", name=f"{name}{kt}"
                    )
                    nc.sync.dma_start(out=t[:], in_=w_dram[name][ts(kt, 128), :])
                    tiles.append(t)
                w_sb[name] = tiles

            load_w("wq")
            bias_sb = {}
            for name, dram in (("bq", bq), ("bk", bk)):
                tiles = []
                for nt in range(2):
                    t = cpool.tile([128, 1], FP, tag=f"{name}{nt}")
                    nc.sync.dma_start(out=t[:], in_=dram[ts(nt, 128), :])
                    tiles.append(t)
                bias_sb[name] = tiles
            # broadcast rows loaded via stride-0 DRAM reads
            bvB = cpool.tile([128, DC], FP, tag="bvB")
            nc.sync.dma_start(out=bvB[:], in_=bv[:].broadcast_to([128, DC]))
            gammaB = cpool.tile([128, H], FP, tag="gammaB")
            nc.sync.dma_start(out=gammaB[:], in_=gamma[:].broadcast_to([128, H]))
            betaB = cpool.tile([128, H], FP, tag="betaB")
            nc.sync.dma_start(out=betaB[:], in_=beta[:].broadcast_to([128, H]))

            # ---------- q/k projections (transposed outputs [DC, S]) ----------
            qhT_sb = [cpool.tile([128, S], BF, tag=f"qhT{nt}", name=f"qhT{nt}") for nt in range(2)]
            khT_sb = [cpool.tile([128, S], BF, tag=f"khT{nt}", name=f"khT{nt}") for nt in range(2)]
            for bname, xdram, outsb in (("bq", qT, qhT_sb), ("bk", kTt, khT_sb)):
                wname = "wq" if bname == "bq" else "wk"
                if wname not in w_sb:
                    load_w(wname)
                x_tiles = []
                for kt in range(KT):
                    xt = spool.tile([128, S], BF, tag=f"x{kt}")
                    nc.sync.dma_start(out=xt[:], in_=xdram[ts(kt, 128), :])
                    x_tiles.append(xt)
                for nt in range(2):
                    for qc in range(QC):
                        ps = ppoolA.tile([128, 512], FP, tag="A")
                        for kt in range(KT):
                            nc.tensor.matmul(
                                ps[:],
                                lhsT=w_sb[wname][kt][:, ts(nt, 128)],
                                rhs=x_tiles[kt][:, ts(qc, 512)],
                                start=(kt == 0),
                                stop=(kt == KT - 1),
                            )
                        nc.any.tensor_scalar_add(
                            outsb[nt][:, ts(qc, 512)], ps[:], bias_sb[bname][nt][:]
                        )

            # ---------- v projection (natural layout -> vext [S, 4*65]) ----------
            vext_sb = []
            for st in range(ST):
                t = cpool.tile([128, HPC * E1], BF, tag=f"vext{st}")
                vext_sb.append(t)
            load_w("wv")
            vT_tiles = []
            for ht in range(KT):
                xt = spool.tile([128, S], BF, tag=f"x{ht}")
                nc.sync.dma_start(out=xt[:], in_=vT[ts(ht, 128), :])
                vT_tiles.append(xt)
            for st in range(ST):
                ps = ppoolA.tile([128, DC], FP, tag="A")
                for ht in range(KT):
                    nc.tensor.matmul(
                        ps[:],
                        lhsT=vT_tiles[ht][:, ts(st, 128)],
                        rhs=w_sb["wv"][ht][:],
                        start=(ht == 0),
                        stop=(ht == KT - 1),
                    )
                vx = vext_sb[st]
                for h in range(HPC):
                    nc.vector.memset(vx[:, h * E1 + HD : h * E1 + E1], 1.0)
                v3 = vx.rearrange("p (h e) -> p h e", e=E1)[:, :, 0:HD]
                p3 = ps.rearrange("p (h e) -> p h e", e=HD)
                b3 = bvB.rearrange("p (h e) -> p h e", e=HD)
                nc.vector.tensor_add(v3, p3, b3)

            # ---------- attention: head pairs x q-halves ----------
            outT_sc = [cpool.tile([128, S], BF, tag=f"oT{nt}", name=f"oT{nt}") for nt in range(2)]
            for hp in range(2):
                for half in range(2):
                    q0 = 1024 * half
                    Bts = [
                        ppool.tile([E1, 1024], FP, tag=f"attB{hh}", name=f"attB{hh}")
                        for hh in range(2)
                    ]
                    for kj in range(ST):
                        mt = mpool.tile([128, 1024], BF, tag="mask")
                        nc.sync.dma_start(
                            out=mt[:], in_=maskT[ts(kj, 128), q0 : q0 + 1024]
                        )
                        for hh in range(2):
                            hb = 64 * hh
                            A = ppoolA.tile([128, 1024], FP, tag="A")
                            for c in range(2):
                                nc.tensor.matmul(
                                    A[:, ts(c, 512)],
                                    lhsT=khT_sb[hp][hb : hb + 64, ts(kj, 128)],
                                    rhs=qhT_sb[hp][hb : hb + 64, q0 + 512 * c : q0 + 512 * (c + 1)],
                                    start=True,
                                    stop=True,
                                )
                            P = wpool.tile([128, 1024], BF, tag="P")
                            nc.scalar.activation(P[:], A[:], AF.Exp, scale=INV_SCALE)
                            Pm = wpool.tile([128, 1024], BF, tag="Pm")
                            nc.vector.tensor_mul(Pm[:], P[:], mt[:])
                            h = 2 * hp + hh
                            for c in range(2):
                                nc.tensor.matmul(
                                    Bts[hh][:, ts(c, 512)],
                                    lhsT=vext_sb[kj][:, h * E1 : (h + 1) * E1],
                                    rhs=Pm[:, ts(c, 512)],
                                    start=(kj == 0),
                                    stop=(kj == ST - 1),
                                )
                    # epilogue: divide rows 0..63 by the ones-row (denominator)
                    for hh in range(2):
                        hb = 64 * hh
                        dn65 = epool.tile([E1, 1024], FP, tag="dn65")
                        nc.any.tensor_copy(dn65[64:65, :], Bts[hh][64:65, :])
                        dnP = epool.tile([128, 8], FP, tag="dnP")
                        nc.sync.dma_start(out=dnP[:], in_=dn65[64:65, :])
                        rcP = epool.tile([128, 8], FP, tag="rcP")
                        nc.vector.reciprocal(rcP[:], dnP[:])
                        rdram = dpool2.tile([1, 1024], FP, tag="rdram")
                        nc.sync.dma_start(out=rdram[:], in_=rcP[:])
                        rb = epool.tile([64, 1024], FP, tag="rb")
                        nc.sync.dma_start(
                            out=rb[:], in_=rdram[:].broadcast_to([64, 1024])
                        )
                        if hh == 0:
                            nc.vector.tensor_mul(
                                outT_sc[hp][0:64, q0 : q0 + 1024],
                                Bts[hh][0:64, :],
                                rb[:],
                            )
                        else:
                            osc = epool.tile([64, 1024], BF, tag="osc")
                            nc.vector.tensor_mul(osc[:], Bts[hh][0:64, :], rb[:])
                            nc.sync.dma_start(
                                out=outT_sc[hp][64:128, q0 : q0 + 1024], in_=osc[:]
                            )

            wfc_sb = []
            for dg in range(2):
                t = cpool.tile([128, H], BF, tag=f"wfc{dg}", name=f"wfc{dg}")
                nc.sync.dma_start(out=t[:], in_=wfc[ts(dg, 128), :])
                wfc_sb.append(t)

            # ---------- fc partial + chunked reduce-scatter ----------
            # y_part chunk i covers s-rows [512i, 512(i+1)); after each chunk's
            # four s-tiles finish, a ReduceScatter over the 4-core group hands
            # this core rows [512i+128r, 512i+128(r+1)) (r = group rank).
            y_chunks = [
                dpool.tile([SR, H], BF, tag=f"y_part{i}", name=f"y_part{i}")
                for i in range(RT)
            ]
            z_chunks = [
                dpool.tile([128, H], BF, tag=f"z{i}", name=f"z{i}")
                for i in range(RT)
            ]
            for st in range(ST):
                ps = ppoolA.tile([128, H], FP, tag="A")
                for dg in range(2):
                    for hc in range(2):
                        nc.tensor.matmul(
                            ps[:, ts(hc, 512)],
                            lhsT=outT_sc[dg][:, ts(st, 128)],
                            rhs=wfc_sb[dg][:, ts(hc, 512)],
                            start=(dg == 0),
                            stop=(dg == 1),
                        )
                yb = fpool.tile([128, H], BF, tag="yb", bufs=4)
                nc.any.tensor_copy(yb[:], ps[:])
                nc.sync.dma_start(
                    out=y_chunks[st // 4][ts(st % 4, 128), :], in_=yb[:]
                )
                if st % 4 == 3:
                    nc.gpsimd.collective_compute(
                        "ReduceScatter",
                        ALU.add,
                        replica_groups=[[0, 1, 2, 3], [4, 5, 6, 7]],
                        ins=[y_chunks[st // 4][:]],
                        outs=[z_chunks[st // 4][:]],
                    )

            # ---------- residual + layernorm on own 4x128-row slices ----------
            for rt in range(RT):
                zbf = fpool.tile([128, H], BF, tag="zbf", bufs=4)
                nc.sync.dma_start(out=zbf[:], in_=z_chunks[rt][:])
                rs = fpool.tile([128, H], FP, tag="rs")
                nc.sync.dma_start(out=rs[:], in_=resid[ts(rt, 128), :])
                musum = fpool.tile([128, 1], FP, tag="musum")
                zt = fpool.tile([128, H], FP, tag="zt")
                # y = z + resid, accumulating row sums for the mean
                nc.vector.scalar_tensor_tensor(
                    zt[:], zbf[:], 0.0, rs[:], ALU.add, ALU.add, accum_out=musum[:]
                )
                nmu = fpool.tile([128, 1], FP, tag="nmu")
                nc.vector.tensor_scalar_mul(nmu[:], musum[:], -1.0 / H)
                nc.scalar.activation(zt[:], zt[:], AF.Identity, bias=nmu[:])
                ssq = fpool.tile([128, 1], FP, tag="ssq")
                nc.vector.scalar_tensor_tensor(
                    rs[:], zt[:], 0.0, zt[:], ALU.add, ALU.mult, accum_out=ssq[:]
                )
                varp = fpool.tile([128, 1], FP, tag="varp")
                nc.vector.tensor_scalar(
                    varp[:], ssq[:], 1.0 / H, EPS, ALU.mult, ALU.add
                )
                sdev = fpool.tile([128, 1], FP, tag="sdev")
                nc.scalar.activation(sdev[:], varp[:], AF.Sqrt)
                rstd = fpool.tile([128, 1], FP, tag="rstd")
                nc.vector.reciprocal(rstd[:], sdev[:])
                nc.vector.scalar_tensor_tensor(
                    rs[:], zt[:], rstd[:], gammaB[:], ALU.mult, ALU.mult
                )
                ot = fpool.tile([128, H], FP, tag="ot")
                nc.vector.tensor_add(ot[:], rs[:], betaB[:])
                nc.sync.dma_start(out=out[ts(rt, 128), :], in_=ot[:])

    nc.compile()
    return nc


_NC_CACHE = {}


def _get_nc():
    if "nc" not in _NC_CACHE:
        _NC_CACHE["nc"] = _build_nc()
    return _NC_CACHE["nc"]


def _prep_inputs(q, k, v, mask, Wq, bq, Wk, bk, Wv, bv, Wfc, bfc, gamma, beta):
    """Build the 8 per-core input maps on the host (sharding + layout)."""
    q = np.asarray(q, F32)
    k = np.asarray(k, F32)
    v = np.asarray(v, F32)
    mask = np.asarray(mask)
    in_maps = []
    qT_b, kT_b, vT_b, maskT_b = [], [], [], []
    for b in range(B):
        qT_b.append(np.ascontiguousarray(q[b].T).astype(BF16))
        kT_b.append(np.ascontiguousarray(k[b].T).astype(BF16))
        vT_b.append(np.ascontiguousarray(v[b].T).astype(BF16))
        maskT_b.append(np.ascontiguousarray(mask[b, 0].T).astype(BF16))
    Wq_bf, Wk_bf, Wv_bf, Wfc_bf = (
        np.asarray(w, F32).astype(BF16) for w in (Wq, Wk, Wv, Wfc)
    )
    for c in range(NCORES):
        b, g = c // TPG, c % TPG
        cols = slice(g * DC, (g + 1) * DC)
        in_maps.append({
            "qT": qT_b[b],
            "kTt": kT_b[b],
            "vT": vT_b[b],
            "maskT": maskT_b[b],
            "wq": np.ascontiguousarray(Wq_bf[:, cols]),
            "wk": np.ascontiguousarray(Wk_bf[:, cols]),
            "wv": np.ascontiguousarray(Wv_bf[:, cols]),
            "wfc": np.ascontiguousarray(Wfc_bf[cols, :]),
            "bq": np.asarray(bq, F32)[cols].reshape(DC, 1),
            "bk": np.asarray(bk, F32)[cols].reshape(DC, 1),
            "bv": np.asarray(bv, F32)[cols].reshape(1, DC),
            "resid": np.ascontiguousarray(
                np.concatenate(
                    [
                        q[b, 512 * i + 128 * g : 512 * i + 128 * (g + 1)]
                        for i in range(RT)
                    ]
                )
                + np.asarray(bfc, F32)[None, :]
            ),
            "gamma": np.asarray(gamma, F32).reshape(1, H),
            "beta": np.asarray(beta, F32).reshape(1, H),
        })
    return in_maps


_LAST_RUN_S = [None]


def kernel(q, k, v, mask, Wq, bq, Wk, bk, Wv, bv, Wfc, bfc, gamma, beta):
    import time

    nc = _get_nc()
    in_maps = _prep_inputs(
        q, k, v, mask, Wq, bq, Wk, bk, Wv, bv, Wfc, bfc, gamma, beta
    )
    t0 = time.perf_counter()
    res = run_bass_kernel_spmd(nc, in_maps, list(range(NCORES)))
    _LAST_RUN_S[0] = time.perf_counter() - t0
    full = np.empty((B, S, H), F32)
    for c in range(NCORES):
        b, r = c // TPG, c % TPG
        o = res.results[c]["out"]
        for i in range(RT):
            full[b, 512 * i + 128 * r : 512 * i + 128 * (r + 1)] = o[
                128 * i : 128 * (i + 1)
            ]
    return full


# revision 22
# speedup vs baseline: 138.4852x; 2.1390x over previous
"""Trainium2 Bass kernel for a fused multi-head attention block.

Reference computation (B=2, S=2048, H=1024, NH=16, HD=64):
    qh/kh/vh = (x @ W + b) per head
    energy   = qh @ kh^T  (full S x S per head)
    attn     = softmax(where(mask==0, -1e9, energy) / sqrt(H))
    out      = attn @ vh
    y        = out @ Wfc + bfc + q (residual)
    return LayerNorm(y) * gamma + beta

Sharding: data-parallel over batch (2 groups of 4 cores) x tensor-parallel
over heads (4 heads per core). Wq/Wk/Wv column-sharded, Wfc row-sharded,
ReduceScatter(add) over each 4-core group after fc, then per-core
residual+LayerNorm on its 512-row output slice.

Per-core kernel layout choices:
  * q/k projections produce TRANSPOSED activations qh^T/kh^T [256, S] so the
    scores matmul (contraction over head dim) can run directly from SBUF.
  * scores are computed transposed: energy^T[k, q] tiles, so exp can be
    applied on the PSUM tile and the softmax denominator comes for free via
    an appended ones-column in the attn@V stationary operand (vext).
  * the masked softmax is multiplicative: P = exp(energy/32) * maskT, which
    is exactly equivalent to the reference's -1e9 additive mask since
    exp(-1e9/32) == 0 in fp32.
  * attn@V accumulates out^T[d|1, q] in PSUM over k-tiles; row 64 holds the
    denominator. The divide is done as reciprocal + broadcast-multiply
    (broadcast via a stride-0 DRAM DMA).
"""

import numpy as np
import ml_dtypes

import concourse.bass as bass
import concourse.mybir as mybir
from concourse import bacc, tile
from concourse.bass_utils import run_bass_kernel_spmd

B, S, H, NH = 2, 2048, 1024, 16
HD = H // NH                  # 64
NCORES = 8
TPG = 4                       # cores per tensor-parallel group
HPC = NH // TPG               # 4 heads per core
DC = HPC * HD                 # 256 head-dims per core
SR = S // TPG                 # 512 output rows per core
INV_SCALE = 1.0 / float(H) ** 0.5   # 1/32
EPS = 1e-5

FP = mybir.dt.float32
BF = mybir.dt.bfloat16
F32 = np.float32
BF16 = ml_dtypes.bfloat16

KT = H // 128                 # 8 contraction tiles for projections
ST = S // 128                 # 16 seq tiles
QC = S // 512                 # 4 q-chunks of 512
RT = SR // 128                # 4 row tiles in the final phase
E1 = HD + 1                   # 65: head dims + ones column

ts = bass.ts
AF = mybir.ActivationFunctionType
ALU = mybir.AluOpType


def _build_nc():
    nc = bacc.Bacc(
        "TRN2",
        target_bir_lowering=False,
        debug=False,
        num_devices=NCORES,
    )

    # ---- per-core DRAM I/O ----
    qT = nc.dram_tensor("qT", [H, S], BF, kind="ExternalInput")
    kTt = nc.dram_tensor("kTt", [H, S], BF, kind="ExternalInput")
    vT = nc.dram_tensor("vT", [H, S], BF, kind="ExternalInput")
    maskT = nc.dram_tensor("maskT", [S, S], BF, kind="ExternalInput")
    wq = nc.dram_tensor("wq", [H, DC], BF, kind="ExternalInput")
    wk = nc.dram_tensor("wk", [H, DC], BF, kind="ExternalInput")
    wv = nc.dram_tensor("wv", [H, DC], BF, kind="ExternalInput")
    wfc = nc.dram_tensor("wfc", [DC, H], BF, kind="ExternalInput")
    bq = nc.dram_tensor("bq", [DC, 1], FP, kind="ExternalInput")
    bk = nc.dram_tensor("bk", [DC, 1], FP, kind="ExternalInput")
    bv = nc.dram_tensor("bv", [1, DC], FP, kind="ExternalInput")
    resid = nc.dram_tensor("resid", [SR, H], FP, kind="ExternalInput")
    gamma = nc.dram_tensor("gamma", [1, H], FP, kind="ExternalInput")
    beta = nc.dram_tensor("beta", [1, H], FP, kind="ExternalInput")
    out = nc.dram_tensor("out", [SR, H], FP, kind="ExternalOutput")

    with tile.TileContext(nc) as tc:
        with (
            tc.tile_pool(name="const", bufs=1) as cpool,
            tc.tile_pool(name="stream", bufs=2) as spool,
            tc.tile_pool(name="mask", bufs=4) as mpool,
            tc.tile_pool(name="work", bufs=4) as wpool,
            tc.tile_pool(name="epi", bufs=2) as epool,
            tc.tile_pool(name="fin", bufs=2) as fpool,
            tc.tile_pool(name="psum", bufs=1, space="PSUM") as ppool,
            tc.tile_pool(name="psA", bufs=2, space="PSUM") as ppoolA,
            tc.tile_pool(name="dram", bufs=1, space="DRAM") as dpool,
            tc.tile_pool(name="dram2", bufs=2, space="DRAM") as dpool2,
        ):
            # ---------- constants ----------
            # weights are loaded lazily right before the phase that uses them
            # so the first projection's inputs aren't queued behind them
            w_dram = {"wq": wq, "wk": wk, "wv": wv}
            w_sb = {}

            def load_w(name):
                tiles = []
                for kt in range(KT):
                    t = cpool.tile(
                        [128, DC], BF, tag=f"{name}{kt}", name=f"{name}{kt}"
                    )
                    nc.sync.dma_start(out=t[:], in_=w_dram[name][ts(kt, 128), :])
                    tiles.append(t)
                w_sb[name] = tiles

            load_w("wq")
            bias_sb = {}
            for name, dram in (("bq", bq), ("bk", bk)):
                tiles = []
                for nt in range(2):
                    t = cpool.tile([128, 1], FP, tag=f"{name}{nt}")
                    nc.sync.dma_start(out=t[:], in_=dram[ts(nt, 128), :])
                    tiles.append(t)
                bias_sb[name] = tiles
            # broadcast rows loaded via stride-0 DRAM reads
            bvB = cpool.tile([128, DC], FP, tag="bvB")
            nc.sync.dma_start(out=bvB[:], in_=bv[:].broadcast_to([128, DC]))
            gammaB = cpool.tile([128, H], FP, tag="gammaB")
            nc.sync.dma_start(out=gammaB[:], in_=gamma[:].broadcast_to([128, H]))
            betaB = cpool.tile([128, H], FP, tag="betaB")
            nc.sync.dma_start(out=betaB[:], in_=beta[:].broadcast_to([128, H]))

            # ---------- q/k projections (transposed outputs [DC, S]) ----------
            qhT_sb = [cpool.tile([128, S], BF, tag=f"qhT{nt}", name=f"qhT{nt}") for nt in range(2)]
            khT_sb = [cpool.tile([128, S], BF, tag=f"khT{nt}", name=f"khT{nt}") for nt in range(2)]
            for bname, xdram, outsb in (("bq", qT, qhT_sb), ("bk", kTt, khT_sb)):
                wname = "wq" if bname == "bq" else "wk"
                if wname not in w_sb:
                    load_w(wname)
                x_tiles = []
                for kt in range(KT):
                    xt = spool.tile([128, S], BF, tag=f"x{kt}")
                    nc.sync.dma_start(out=xt[:], in_=xdram[ts(kt, 128), :])
                    x_tiles.append(xt)
                for nt in range(2):
                    for qc in range(QC):
                        ps = ppoolA.tile([128, 512], FP, tag="A")
                        for kt in range(KT):
                            nc.tensor.matmul(
                                ps[:],
                                lhsT=w_sb[wname][kt][:, ts(nt, 128)],
                                rhs=x_tiles[kt][:, ts(qc, 512)],
                                start=(kt == 0),
                                stop=(kt == KT - 1),
                            )
                        nc.any.tensor_scalar_add(
                            outsb[nt][:, ts(qc, 512)], ps[:], bias_sb[bname][nt][:]
                        )

            # ---------- v projection (natural layout -> vext [S, 4*65]) ----------
            vext_sb = []
            for st in range(ST):
                t = cpool.tile([128, HPC * E1], BF, tag=f"vext{st}")
                vext_sb.append(t)
            load_w("wv")
            vT_tiles = []
            for ht in range(KT):
                xt = spool.tile([128, S], BF, tag=f"x{ht}")
                nc.sync.dma_start(out=xt[:], in_=vT[ts(ht, 128), :])
                vT_tiles.append(xt)
            for st in range(ST):
                ps = ppoolA.tile([128, DC], FP, tag="A")
                for ht in range(KT):
                    nc.tensor.matmul(
                        ps[:],
                        lhsT=vT_tiles[ht][:, ts(st, 128)],
                        rhs=w_sb["wv"][ht][:],
                        start=(ht == 0),
                        stop=(ht == KT - 1),
                    )
                vx = vext_sb[st]
                for h in range(HPC):
                    nc.vector.memset(vx[:, h * E1 + HD : h * E1 + E1], 1.0)
                v3 = vx.rearrange("p (h e) -> p h e", e=E1)[:, :, 0:HD]
                p3 = ps.rearrange("p (h e) -> p h e", e=HD)
                b3 = bvB.rearrange("p (h e) -> p h e", e=HD)
                nc.vector.tensor_add(v3, p3, b3)

            # ---------- attention: head pairs x q-halves ----------
            outT_sc = [cpool.tile([128, S], BF, tag=f"oT{nt}", name=f"oT{nt}") for nt in range(2)]
            for hp in range(2):
                for half in range(2):
                    q0 = 1024 * half
                    Bts = [
                        ppool.tile([E1, 1024], FP, tag=f"attB{hh}", name=f"attB{hh}")
                        for hh in range(2)
                    ]
                    for kj in range(ST):
                        mt = mpool.tile([128, 1024], BF, tag="mask")
                        nc.sync.dma_start(
                            out=mt[:], in_=maskT[ts(kj, 128), q0 : q0 + 1024]
                        )
                        for hh in range(2):
                            hb = 64 * hh
                            A = ppoolA.tile([128, 1024], FP, tag="A")
                            for c in range(2):
                                nc.tensor.matmul(
                                    A[:, ts(c, 512)],
                                    lhsT=khT_sb[hp][hb : hb + 64, ts(kj, 128)],
                                    rhs=qhT_sb[hp][hb : hb + 64, q0 + 512 * c : q0 + 512 * (c + 1)],
                                    start=True,
                                    stop=True,
                                )
                            P = wpool.tile([128, 1024], BF, tag="P")
                            nc.scalar.activation(P[:], A[:], AF.Exp, scale=INV_SCALE)
                            Pm = wpool.tile([128, 1024], BF, tag="Pm")
                            nc.vector.tensor_mul(Pm[:], P[:], mt[:])
                            h = 2 * hp + hh
                            for c in range(2):
                                nc.tensor.matmul(
                                    Bts[hh][:, ts(c, 512)],
                                    lhsT=vext_sb[kj][:, h * E1 : (h + 1) * E1],
                                    rhs=Pm[:, ts(c, 512)],
                                    start=(kj == 0),
                                    stop=(kj == ST - 1),
                                )
                    # epilogue: divide rows 0..63 by the ones-row (denominator)
                    for hh in range(2):
                        hb = 64 * hh
                        dn65 = epool.tile([E1, 1024], FP, tag="dn65")
                        nc.any.tensor_copy(dn65[64:65, :], Bts[hh][64:65, :])
                        dnP = epool.tile([128, 8], FP, tag="dnP")
                        nc.sync.dma_start(out=dnP[:], in_=dn65[64:65, :])
                        rcP = epool.tile([128, 8], FP, tag="rcP")
                        nc.vector.reciprocal(rcP[:], dnP[:])
                        rdram = dpool2.tile([1, 1024], FP, tag="rdram")
                        nc.sync.dma_start(out=rdram[:], in_=rcP[:])
                        rb = epool.tile([64, 1024], FP, tag="rb")
                        nc.sync.dma_start(
                            out=rb[:], in_=rdram[:].broadcast_to([64, 1024])
                        )
                        if hh == 0:
                            nc.vector.tensor_mul(
                                outT_sc[hp][0:64, q0 : q0 + 1024],
                                Bts[hh][0:64, :],
                                rb[:],
                            )
                        else:
                            osc = epool.tile([64, 1024], BF, tag="osc")
                            nc.vector.tensor_mul(osc[:], Bts[hh][0:64, :], rb[:])
                            nc.sync.dma_start(
                                out=outT_sc[hp][64:128, q0 : q0 + 1024], in_=osc[:]
                            )

            wfc_sb = []
            for dg in range(2):
                t = cpool.tile([128, H], BF, tag=f"wfc{dg}", name=f"wfc{dg}")
                nc.sync.dma_start(out=t[:], in_=wfc[ts(dg, 128), :])
                wfc_sb.append(t)

            # ---------- fc partial + chunked reduce-scatter ----------
            # y_part chunk i covers s-rows [512i, 512(i+1)); after each chunk's
            # four s-tiles finish, a ReduceScatter over the 4-core group hands
            # this core rows [512i+128r, 512i+128(r+1)) (r = group rank).
            y_chunks = [
                dpool.tile([SR, H], BF, tag=f"y_part{i}", name=f"y_part{i}")
                for i in range(RT)
            ]
            z_chunks = [
                dpool.tile([128, H], BF, tag=f"z{i}", name=f"z{i}")
                for i in range(RT)
            ]
            for st in range(ST):
                ps = ppoolA.tile([128, H], FP, tag="A")
                for dg in range(2):
                    for hc in range(2):
                        nc.tensor.matmul(
                            ps[:, ts(hc, 512)],
                            lhsT=outT_sc[dg][:, ts(st, 128)],
                            rhs=wfc_sb[dg][:, ts(hc, 512)],
                            start=(dg == 0),
                            stop=(dg == 1),
                        )
                yb = fpool.tile([128, H], BF, tag="yb", bufs=4)
                nc.any.tensor_copy(yb[:], ps[:])
                nc.sync.dma_start(
                    out=y_chunks[st // 4][ts(st % 4, 128), :], in_=yb[:]
                )
                if st % 4 == 3:
                    nc.gpsimd.collective_compute(
                        "ReduceScatter",
                        ALU.add,
                        replica_groups=[[0, 1, 2, 3], [4, 5, 6, 7]],
                        ins=[y_chunks[st // 4][:]],
                        outs=[z_chunks[st // 4][:]],
                    )

            # ---------- residual + layernorm on own 4x128-row slices ----------
            for rt in range(RT):
                zbf = fpool.tile([128, H], BF, tag="zbf", bufs=4)
                nc.sync.dma_start(out=zbf[:], in_=z_chunks[rt][:])
                rs = fpool.tile([128, H], FP, tag="rs")
                nc.sync.dma_start(out=rs[:], in_=resid[ts(rt, 128), :])
                musum = fpool.tile([128, 1], FP, tag="musum")
                zt = fpool.tile([128, H], FP, tag="zt")
                # y = z + resid, accumulating row sums for the mean
                nc.vector.scalar_tensor_tensor(
                    zt[:], zbf[:], 0.0, rs[:], ALU.add, ALU.add, accum_out=musum[:]
                )
                nmu = fpool.tile([128, 1], FP, tag="nmu")
                nc.vector.tensor_scalar_mul(nmu[:], musum[:], -1.0 / H)
                nc.scalar.activation(zt[:], zt[:], AF.Identity, bias=nmu[:])
                ssq = fpool.tile([128, 1], FP, tag="ssq")
                nc.vector.scalar_tensor_tensor(
                    rs[:], zt[:], 0.0, zt[:], ALU.add, ALU.mult, accum_out=ssq[:]
                )
                varp = fpool.tile([128, 1], FP, tag="varp")
                nc.vector.tensor_scalar(
                    varp[:], ssq[:], 1.0 / H, EPS, ALU.mult, ALU.add
                )
                sdev = fpool.tile([128, 1], FP, tag="sdev")
                nc.scalar.activation(sdev[:], varp[:], AF.Sqrt)
                rstd = fpool.tile([128, 1], FP, tag="rstd")
                nc.vector.reciprocal(rstd[:], sdev[:])
                nc.vector.scalar_tensor_tensor(
                    rs[:], zt[:], rstd[:], gammaB[:], ALU.mult, ALU.mult
                )
                ot = fpool.tile([128, H], FP, tag="ot")
                nc.vector.tensor_add(ot[:], rs[:], betaB[:])
                nc.sync.dma_start(out=out[ts(rt, 128), :], in_=ot[:])

    nc.compile()
    return nc


_NC_CACHE = {}


def _get_nc():
    if "nc" not in _NC_CACHE:
        _NC_CACHE["nc"] = _build_nc()
    return _NC_CACHE["nc"]


def _prep_inputs(q, k, v, mask, Wq, bq, Wk, bk, Wv, bv, Wfc, bfc, gamma, beta):
    """Build the 8 per-core input maps on the host (sharding + layout)."""
    q = np.asarray(q, F32)
    k = np.asarray(k, F32)
    v = np.asarray(v, F32)
    mask = np.asarray(mask)
    in_maps = []
    qT_b, kT_b, vT_b, maskT_b = [], [], [], []
    for b in range(B):
        qT_b.append(np.ascontiguousarray(q[b].T).astype(BF16))
        kT_b.append(np.ascontiguousarray(k[b].T).astype(BF16))
        vT_b.append(np.ascontiguousarray(v[b].T).astype(BF16))
        maskT_b.append(np.ascontiguousarray(mask[b, 0].T).astype(BF16))
    Wq_bf, Wk_bf, Wv_bf, Wfc_bf = (
        np.asarray(w, F32).astype(BF16) for w in (Wq, Wk, Wv, Wfc)
    )
    for c in range(NCORES):
        b, g = c // TPG, c % TPG
        cols = slice(g * DC, (g + 1) * DC)
        in_maps.append({
            "qT": qT_b[b],
            "kTt": kT_b[b],
            "vT": vT_b[b],
            "maskT": maskT_b[b],
            "wq": np.ascontiguousarray(Wq_bf[:, cols]),
            "wk": np.ascontiguousarray(Wk_bf[:, cols]),
            "wv": np.ascontiguousarray(Wv_bf[:, cols]),
            "wfc": np.ascontiguousarray(Wfc_bf[cols, :]),
            "bq": np.asarray(bq, F32)[cols].reshape(DC, 1),
            "bk": np.asarray(bk, F32)[cols].reshape(DC, 1),
            "bv": np.asarray(bv, F32)[cols].reshape(1, DC),
            "resid": np.ascontiguousarray(
                np.concatenate(
                    [
                        q[b, 512 * i + 128 * g : 512 * i + 128 * (g + 1)]
                        for i in range(RT)
                    ]
                )
                + np.asarray(bfc, F32)[None, :]
            ),
            "gamma": np.asarray(gamma, F32).reshape(1, H),
            "beta": np.asarray(beta, F32).reshape(1, H),
        })
    return in_maps


_LAST_RUN_S = [None]


def kernel(q, k, v, mask, Wq, bq, Wk, bk, Wv, bv, Wfc, bfc, gamma, beta):
    import time

    nc = _get_nc()
    in_maps = _prep_inputs(
        q, k, v, mask, Wq, bq, Wk, bk, Wv, bv, Wfc, bfc, gamma, beta
    )
    t0 = time.perf_counter()
    res = run_bass_kernel_spmd(nc, in_maps, list(range(NCORES)))
    _LAST_RUN_S[0] = time.perf_counter() - t0
    full = np.empty((B, S, H), F32)
    for c in range(NCORES):
        b, r = c // TPG, c % TPG
        o = res.results[c]["out"]
        for i in range(RT):
            full[b, 512 * i + 128 * r : 512 * i + 128 * (r + 1)] = o[
                128 * i : 128 * (i + 1)
            ]
    return full
